# revision 19
# baseline (speedup 1.0000x reference)
"""Trainium2 Bass kernel for nn_ExplicitRegisters (scatter_memory), v2.1.

Reference math (per batch, L tokens, dim D, K heads, R registers):
    h   = LN(x) * g + b
    rw  = softmax(h @ rq_w + rq_b);  ww = softmax(h @ wq_w + wq_b)
    wv  = h @ wv_w + wv_b;           wg = sigmoid(h @ wg_w + wg_b)
    us  = ww * wg
    scan: rv_t = sum_k rw[t,k] regs[k,r]  (read before write)
          regs = (1-us_t) regs + us_t wv_t
    out = mix * (rv @ rp_w + rp_b)

v2 strategy (pure data parallel, one batch element per NeuronCore):
  - Host ships x twice in bf16, pre-swizzled so every input DMA is 128
    partitions x 8KB contiguous descriptors: token-major (LN stats) and
    d-major (matmul moving operand). Output goes to a permuted DRAM
    buffer (same descriptor shape) and is unpermuted on the host.
  - Fully block-pipelined program (4 blocks of 512 tokens).
  - One padded bf16 matmul computes all projections channel-major:
    z^T rows [0:32)=wv, [32:40)=rq, [64:72)=wq, 96=gate.
  - LN folded post-matmul via rank-1 PE corrections; per-block stats
    via bn_stats (DVE) + accum passes (ACT), row-form via one tiny PE
    transpose per block + SBUF DMA rearrange.
  - Unnormalized softmax; write normalization folded into
    alpha = sigmoid(gate)/sum_k E_w (reciprocal_approx_fast); read
    normalization applied at the output tiles via appended sum row.
  - Recurrence: native tensor_tensor_scan over 256 (k,r) lanes in two
    [128, L] tiles.
"""

import os
import numpy as np
import ml_dtypes

import concourse.bacc as bacc
import concourse.bass as bass
import concourse.tile as tile
from concourse import mybir
from concourse.bass_utils import run_bass_kernel_spmd

B, L, D, K, R = 8, 2048, 1024, 8, 32
NCORES = 8
P = 128
T = L
NT = T // P            # 16 token tiles
NB = 4                 # blocks
BLK = 512
TPB = BLK // P         # 4 token tiles per block
ND = D // P            # 8 d-slices
EPS = 1e-5

# padded channel layout of the z matmul (partition bases must be 32-aligned)
C_WV = 0               # rows 0..31
C_RQ = 32              # rows 32..39
C_WQ = 64              # rows 64..71
C_G = 96               # row 96

f32 = mybir.dt.float32
f32r = mybir.dt.float32r
bf16 = mybir.dt.bfloat16
ts = bass.ts
AF = mybir.ActivationFunctionType
OP = mybir.AluOpType


def _r(ap):
    return ap if ap.dtype == f32r else ap.bitcast(f32r)


def build_program(wgb_eff: float, use_bias_vec: bool, debug: bool = False):
    nc = bacc.Bacc("TRN2", target_bir_lowering=False, debug=False,
                   enable_asserts=False, num_devices=NCORES)

    # pre-swizzled inputs: [P, ...] partition-major, 8KB/partition/block
    xbi_d = nc.dram_tensor("xbi", [P, NT, D], bf16, kind="ExternalInput")
    xTi_d = nc.dram_tensor("xTi", [P, NB, ND, BLK], bf16, kind="ExternalInput")
    wg_d = nc.dram_tensor("wg", [D, P], bf16, kind="ExternalInput")
    rp_d = nc.dram_tensor("rp", [R + 2, D], f32r, kind="ExternalInput")
    negs_d = nc.dram_tensor("negs", [1, P], f32r, kind="ExternalInput")
    ident_f_d = nc.dram_tensor("ident_f", [P, P], f32, kind="ExternalInput")
    nselK0_d = nc.dram_tensor("nselK0", [K + 1, P], f32r, kind="ExternalInput")
    nselK1_d = nc.dram_tensor("nselK1", [K + 1, P], f32r, kind="ExternalInput")
    selK0_d = nc.dram_tensor("selK0", [K, P], f32r, kind="ExternalInput")
    selK1_d = nc.dram_tensor("selK1", [K, P], f32r, kind="ExternalInput")
    selR_d = nc.dram_tensor("selR", [R, P], f32r, kind="ExternalInput")
    selO_d = nc.dram_tensor("selO", [P, R + 2], f32r, kind="ExternalInput")
    e32_d = nc.dram_tensor("e32", [K, R + 2], f32r, kind="ExternalInput")
    onesP_d = nc.dram_tensor("onesP", [1, P], f32r, kind="ExternalInput")
    ones8c_d = nc.dram_tensor("ones8c", [K, 2], f32r, kind="ExternalInput")
    ones1x8_d = nc.dram_tensor("ones1x8", [1, K], f32r, kind="ExternalInput")
    onesrow_d = nc.dram_tensor("onesrow", [1, T], f32r, kind="ExternalInput")
    bvec_d = nc.dram_tensor("bvec", [P, 1], f32, kind="ExternalInput")
    # permuted output: y_perm[p, i, d] = y[i*128 + p, d]
    y_d = nc.dram_tensor("y", [P, NT, D], f32, kind="ExternalOutput")
    if debug:
        dbg = {
            "d_mv": nc.dram_tensor("d_mv", [P, NT, 2], f32, kind="ExternalOutput"),
            "d_musrow": nc.dram_tensor("d_musrow", [1, T], f32, kind="ExternalOutput"),
            "d_rstdrow": nc.dram_tensor("d_rstdrow", [1, T], f32, kind="ExternalOutput"),
            "d_zlnT": nc.dram_tensor("d_zlnT", [P, T], f32, kind="ExternalOutput"),
            "d_ErT": nc.dram_tensor("d_ErT", [K, T], f32, kind="ExternalOutput"),
            "d_EwT": nc.dram_tensor("d_EwT", [K, T], f32, kind="ExternalOutput"),
            "d_grow": nc.dram_tensor("d_grow", [1, T], f32, kind="ExternalOutput"),
            "d_usT": nc.dram_tensor("d_usT", [K + 1, T], f32, kind="ExternalOutput"),
            "d_scan0": nc.dram_tensor("d_scan0", [P, T + 1], f32, kind="ExternalOutput"),
            "d_scan1": nc.dram_tensor("d_scan1", [P, T + 1], f32, kind="ExternalOutput"),
            "d_rvT": nc.dram_tensor("d_rvT", [R + 2, T], f32, kind="ExternalOutput"),
            "d_rsr": nc.dram_tensor("d_rsr", [P, NT], f32, kind="ExternalOutput"),
        }

    yap = y_d.ap()

    with tile.TileContext(nc) as tc:
        with (
            tc.tile_pool(name="consts", bufs=1) as consts,
            tc.tile_pool(name="big", bufs=1) as big,
            tc.tile_pool(name="blk", bufs=2) as blk,
            tc.tile_pool(name="ps_z", bufs=1, space="PSUM") as ps_z,
            tc.tile_pool(name="ps_mm", bufs=2, space="PSUM") as ps_mm,
            tc.tile_pool(name="ps_pre", bufs=1, space="PSUM") as ps_pre,
            tc.tile_pool(name="ps_post", bufs=1, space="PSUM") as ps_post,
            tc.tile_pool(name="ps_rv", bufs=1, space="PSUM") as ps_rv,
            tc.tile_pool(name="ps_y", bufs=2, space="PSUM") as ps_y,
        ):
            # ---- persistent state ----
            xb_s = big.tile([P, NT, D], bf16)
            xT_s = big.tile([P, NB, ND, BLK], bf16)
            zlnT = big.tile([P, T], f32r)
            ErT = big.tile([K, T], f32r)
            EwT = big.tile([K, T], f32r)
            usT = big.tile([K + 1, T], f32r)     # row 8 = ones
            grow = big.tile([1, T], f32r)        # gate -> egn
            musrow = big.tile([1, T], f32r)
            rstdrow = big.tile([1, T], f32r)
            mv = big.tile([P, NT, 2], f32)       # (mean, var) per token
            scan0 = big.tile([P, T + 1], f32)
            scan1 = big.tile([P, T + 1], f32)
            rvT = big.tile([R + 2, T], f32r)
            rsr = big.tile([P, NT], f32)

            # ---- constants (wg first: z matmul needs it earliest) ----
            wg_s = consts.tile([P, ND, P], bf16)
            nc.sync.dma_start(out=wg_s,
                              in_=wg_d.ap().rearrange("(j p) c -> p j c", p=P))

            # block 0 inputs before the remaining consts
            nc.sync.dma_start(out=xT_s[:, 0], in_=xTi_d.ap()[:, 0])
            nc.sync.dma_start(out=xb_s[:, 0:TPB, :],
                              in_=xbi_d.ap()[:, 0:TPB, :])

            rp_s = consts.tile([R + 2, D], f32r)
            nc.sync.dma_start(out=rp_s, in_=rp_d.ap())
            ident_f = consts.tile([P, P], f32)
            nc.sync.dma_start(out=ident_f, in_=ident_f_d.ap())
            nselK0 = consts.tile([K + 1, P], f32r)
            nc.sync.dma_start(out=nselK0, in_=nselK0_d.ap())
            nselK1 = consts.tile([K + 1, P], f32r)
            nc.sync.dma_start(out=nselK1, in_=nselK1_d.ap())
            selK0 = consts.tile([K, P], f32r)
            nc.sync.dma_start(out=selK0, in_=selK0_d.ap())
            selK1 = consts.tile([K, P], f32r)
            nc.sync.dma_start(out=selK1, in_=selK1_d.ap())
            selR = consts.tile([R, P], f32r)
            nc.sync.dma_start(out=selR, in_=selR_d.ap())
            selO = consts.tile([P, R + 2], f32r)
            nc.sync.dma_start(out=selO, in_=selO_d.ap())
            e32 = consts.tile([K, R + 2], f32r)
            nc.sync.dma_start(out=e32, in_=e32_d.ap())
            negs = consts.tile([1, P], f32r)
            nc.sync.dma_start(out=negs, in_=negs_d.ap())
            onesP = consts.tile([1, P], f32r)
            nc.sync.dma_start(out=onesP, in_=onesP_d.ap())
            ones8c = consts.tile([K, 2], f32r)
            nc.sync.dma_start(out=ones8c, in_=ones8c_d.ap())
            ones1x8 = consts.tile([1, K], f32r)
            nc.sync.dma_start(out=ones1x8, in_=ones1x8_d.ap())
            bvec = consts.tile([P, 1], f32)
            nc.sync.dma_start(out=bvec, in_=bvec_d.ap())
            epsb = consts.tile([P, 1], f32)
            nc.vector.memset(epsb, EPS)
            gbias = consts.tile([P, 1], f32)
            nc.vector.memset(gbias, -wgb_eff)

            nc.vector.memset(scan0[:, 0:1], 0.0)
            nc.vector.memset(scan1[:, 0:1], 0.0)
            nc.sync.dma_start(out=usT[K:K + 1, :], in_=onesrow_d.ap())

            # remaining input blocks
            for b in range(1, NB):
                nc.sync.dma_start(out=xT_s[:, b], in_=xTi_d.ap()[:, b])
                nc.sync.dma_start(out=xb_s[:, b * TPB:(b + 1) * TPB, :],
                                  in_=xbi_d.ap()[:, b * TPB:(b + 1) * TPB, :])

            # ---- per-block pipeline ----
            for b in range(NB):
                # -- stats: 2 tiles DVE bn_stats, 2 tiles ACT copy/square --
                for ii in range(TPB):
                    i = b * TPB + ii
                    xi = xb_s[:, i, :]
                    if ii < 2:
                        st6 = blk.tile([P, 2, 6], f32, tag="st6")
                        xig = xi.rearrange("p (g f) -> p g f", f=512)
                        nc.vector.bn_stats(st6[:, 0, :], xig[:, 0, :])
                        nc.vector.bn_stats(st6[:, 1, :], xig[:, 1, :])
                        nc.vector.bn_aggr(mv[:, i, :], st6)
                    else:
                        sc = blk.tile([P, D], bf16, tag="sc")
                        scol = blk.tile([P, 2], f32, tag="scol")
                        nc.scalar.activation(sc, xi, AF.Copy,
                                             accum_out=scol[:, 0:1])
                        nc.scalar.activation(sc, xi, AF.Square,
                                             accum_out=scol[:, 1:2])
                        nc.vector.tensor_scalar(
                            out=mv[:, i, 0:1], in0=scol[:, 0:1],
                            scalar1=1.0 / D, scalar2=None, op0=OP.mult)
                        msq = blk.tile([P, 1], f32, tag="msq")
                        nc.vector.tensor_mul(msq, mv[:, i, 0:1], mv[:, i, 0:1])
                        nc.vector.scalar_tensor_tensor(
                            out=mv[:, i, 1:2], in0=scol[:, 1:2],
                            scalar=1.0 / D, in1=msq,
                            op0=OP.mult, op1=OP.subtract)

                # -- z matmul (held open for the mu correction) --
                zp = ps_z.tile([P, BLK], f32, tag="zp")
                for j in range(ND):
                    nc.tensor.matmul(zp, wg_s[:, j, :],
                                     xT_s[:, b, j, :],
                                     start=(j == 0), stop=False)

                # -- stats tail: column math + row-form conversion --
                pack = blk.tile([P, 2 * TPB], f32, tag="pack")
                nc.vector.tensor_copy(pack[:, 0:TPB],
                                      mv[:, b * TPB:(b + 1) * TPB, 0])
                lnv = blk.tile([P, TPB], f32, tag="lnv")
                nc.scalar.activation(lnv, mv[:, b * TPB:(b + 1) * TPB, 1],
                                     AF.Ln, bias=epsb)
                nc.scalar.activation(pack[:, TPB:2 * TPB], lnv, AF.Exp,
                                     scale=-0.5)
                pkT = ps_pre.tile([2 * TPB, P], f32, tag="pre")
                nc.tensor.transpose(pkT, pack, ident_f)
                pks = blk.tile([2 * TPB, P], f32r, tag="pks")
                nc.vector.tensor_copy(pks, pkT)
                nc.gpsimd.dma_start(
                    out=musrow[:, ts(b, BLK)].rearrange(
                        "o (c p) -> o c p", p=P),
                    in_=pks[0:TPB, :])
                nc.gpsimd.dma_start(
                    out=rstdrow[:, ts(b, BLK)].rearrange(
                        "o (c p) -> o c p", p=P),
                    in_=pks[TPB:2 * TPB, :])

                # -- LN fold: rank-1 mu correction + rstd scale --
                nc.tensor.matmul(
                    zp, _r(negs), _r(musrow[:, ts(b, BLK)]),
                    start=False, stop=True, skip_group_check=True)
                rr = ps_mm.tile([P, BLK], f32, tag="mm")
                nc.tensor.matmul(rr, _r(onesP), _r(rstdrow[:, ts(b, BLK)]),
                                 start=True, stop=True)
                rrs = blk.tile([P, BLK], f32, tag="rrs")
                nc.scalar.copy(rrs, rr)
                nc.vector.tensor_mul(zlnT[:, ts(b, BLK)], zp, rrs)
                if use_bias_vec:
                    nc.vector.tensor_scalar(
                        out=zlnT[:, ts(b, BLK)], in0=zlnT[:, ts(b, BLK)],
                        scalar1=bvec[:, 0:1], scalar2=None, op0=OP.add)

                # -- row extractions + exp --
                nc.gpsimd.dma_start(out=ErT[:, ts(b, BLK)],
                                    in_=zlnT[C_RQ:C_RQ + K, ts(b, BLK)])
                nc.gpsimd.dma_start(out=EwT[:, ts(b, BLK)],
                                    in_=zlnT[C_WQ:C_WQ + K, ts(b, BLK)])
                nc.gpsimd.dma_start(out=grow[:, ts(b, BLK)],
                                    in_=zlnT[C_G:C_G + 1, ts(b, BLK)])
                nc.scalar.activation(ErT[:, ts(b, BLK)], ErT[:, ts(b, BLK)],
                                     AF.Exp)
                nc.scalar.activation(EwT[:, ts(b, BLK)], EwT[:, ts(b, BLK)],
                                     AF.Exp)
                # egn = exp(-(g + wgb))
                nc.scalar.activation(grow[:, ts(b, BLK)], grow[:, ts(b, BLK)],
                                     AF.Exp, scale=-1.0, bias=gbias[0:1, 0:1])

                # -- alpha = sigmoid(g)/sum_k E_w, broadcast to 8 rows --
                swp = ps_pre.tile([2, BLK], f32, tag="pre")
                nc.tensor.matmul(swp, ones8c, _r(EwT[:, ts(b, BLK)]),
                                 start=True, stop=True)
                deno = blk.tile([1, BLK], f32r, tag="deno")
                nc.vector.scalar_tensor_tensor(
                    out=deno, in0=grow[:, ts(b, BLK)], scalar=1.0,
                    in1=swp[0:1, :], op0=OP.add, op1=OP.mult)
                den8 = ps_pre.tile([K, BLK], f32, tag="pre")
                nc.tensor.matmul(den8, _r(ones1x8), deno,
                                 start=True, stop=True)
                rd8 = blk.tile([K, BLK], f32, tag="rd8")
                nc.vector.reciprocal_approx_fast(rd8, den8)
                nc.vector.tensor_mul(usT[0:K, ts(b, BLK)],
                                     EwT[:, ts(b, BLK)], rd8)

                # -- replicate + scan --
                for kt, (nselK, selK, scanb) in enumerate(
                        [(nselK0, selK0, scan0), (nselK1, selK1, scan1)]):
                    d0p = ps_mm.tile([P, BLK], f32, tag="mm")
                    nc.tensor.matmul(d0p, _r(nselK), _r(usT[:, ts(b, BLK)]),
                                     start=True, stop=True)
                    wrep = ps_mm.tile([P, BLK], f32, tag="mm")
                    nc.tensor.matmul(wrep, _r(selR),
                                     _r(zlnT[C_WV:C_WV + R, ts(b, BLK)]),
                                     start=True, stop=True)
                    wreps = blk.tile([P, BLK], f32, tag="wreps")
                    nc.scalar.copy(wreps, wrep)
                    urep = ps_mm.tile([P, BLK], f32, tag="mm")
                    nc.tensor.matmul(urep, _r(selK),
                                     _r(usT[0:K, ts(b, BLK)]),
                                     start=True, stop=True)
                    d1 = blk.tile([P, BLK], f32, tag="d1")
                    nc.vector.tensor_mul(d1, urep, wreps)
                    nc.vector.tensor_tensor_scan(
                        out=scanb[:, 1 + b * BLK:1 + (b + 1) * BLK],
                        data0=d0p, data1=d1,
                        initial=scanb[:, b * BLK:b * BLK + 1],
                        op0=OP.mult, op1=OP.add)

                # -- read side --
                rvp = ps_rv.tile([R + 2, BLK], f32, tag="rv")
                for kt, (selK, scanb) in enumerate(
                        [(selK0, scan0), (selK1, scan1)]):
                    erep = ps_mm.tile([P, BLK], f32, tag="mm")
                    nc.tensor.matmul(erep, _r(selK), _r(ErT[:, ts(b, BLK)]),
                                     start=True, stop=True)
                    rtmp = blk.tile([P, BLK], f32r, tag="rtmp")
                    nc.vector.tensor_mul(
                        rtmp, scanb[:, ts(b, BLK)], erep)
                    nc.tensor.matmul(rvp, _r(selO), _r(rtmp),
                                     start=(kt == 0), stop=False)
                nc.tensor.matmul(rvp, _r(e32), _r(ErT[:, ts(b, BLK)]),
                                 start=False, stop=True)
                nc.scalar.copy(rvT[:, ts(b, BLK)], rvp)

                # -- read normalization columns: 1/sum_k E_r --
                srt = blk.tile([TPB, P], f32r, tag="srt")
                nc.gpsimd.dma_start(
                    out=srt,
                    in_=rvT[R:R + 1, ts(b, BLK)].rearrange(
                        "o (c p) -> o c p", p=P))
                srtp = ps_post.tile([P, TPB], f32, tag="post")
                nc.tensor.transpose(srtp, srt.bitcast(f32),
                                    ident_f[0:TPB, 0:TPB])
                nc.vector.reciprocal(rsr[:, b * TPB:(b + 1) * TPB], srtp)

                # -- output projection --
                for hh in range(2):
                    ys = blk.tile([P, 2, D], f32, tag="ys")
                    for ii in range(2):
                        i = b * TPB + hh * 2 + ii
                        for half in range(2):
                            yp = ps_y.tile([P, BLK], f32, tag="y")
                            nc.tensor.matmul(yp, _r(rvT[:, ts(i, P)]),
                                             _r(rp_s[:, ts(half, BLK)]),
                                             start=True, stop=True)
                            if (ii + half) % 2 == 0:
                                nc.vector.tensor_scalar(
                                    out=ys[:, ii, ts(half, BLK)], in0=yp,
                                    scalar1=rsr[:, i:i + 1], scalar2=None,
                                    op0=OP.mult)
                            else:
                                nc.scalar.activation(
                                    ys[:, ii, ts(half, BLK)], yp, AF.Copy,
                                    scale=rsr[:, i:i + 1])
                    nc.sync.dma_start(
                        out=yap[:, b * TPB + hh * 2:b * TPB + hh * 2 + 2, :],
                        in_=ys)

            if debug:
                nc.sync.dma_start(out=dbg["d_mv"].ap(), in_=mv)
                nc.sync.dma_start(out=dbg["d_musrow"].ap(), in_=musrow.bitcast(f32))
                nc.sync.dma_start(out=dbg["d_rstdrow"].ap(), in_=rstdrow.bitcast(f32))
                nc.sync.dma_start(out=dbg["d_zlnT"].ap(), in_=zlnT.bitcast(f32))
                nc.sync.dma_start(out=dbg["d_ErT"].ap(), in_=ErT.bitcast(f32))
                nc.sync.dma_start(out=dbg["d_EwT"].ap(), in_=EwT.bitcast(f32))
                nc.sync.dma_start(out=dbg["d_grow"].ap(), in_=grow.bitcast(f32))
                nc.sync.dma_start(out=dbg["d_usT"].ap(), in_=usT.bitcast(f32))
                nc.sync.dma_start(out=dbg["d_scan0"].ap(), in_=scan0)
                nc.sync.dma_start(out=dbg["d_scan1"].ap(), in_=scan1)
                nc.sync.dma_start(out=dbg["d_rvT"].ap(), in_=rvT.bitcast(f32))
                nc.sync.dma_start(out=dbg["d_rsr"].ap(), in_=rsr)

    nc.finalize()
    return nc


_CACHE = {}


def _get_program(wgb_eff: float, use_bias_vec: bool):
    dbgflag = bool(int(os.environ.get("KERNEL_DEBUG", "0")))
    key = (round(float(wgb_eff), 8), use_bias_vec, dbgflag)
    if key not in _CACHE:
        _CACHE[key] = build_program(wgb_eff, use_bias_vec, debug=dbgflag)
    return _CACHE[key]


def kernel(x, ln_g, ln_b, rq_w, rq_b, rp_w, rp_b, wq_w, wq_b, wv_w, wv_b,
           wg_w, wg_b, mix, **_unused):
    x = np.asarray(x, np.float32)
    ln_g = np.asarray(ln_g, np.float32)
    ln_b = np.asarray(ln_b, np.float32)
    mix = np.float32(np.asarray(mix))

    Wfull = np.zeros((D, P), np.float32)
    Wfull[:, C_WV:C_WV + R] = np.asarray(wv_w, np.float32)
    Wfull[:, C_RQ:C_RQ + K] = np.asarray(rq_w, np.float32)
    Wfull[:, C_WQ:C_WQ + K] = np.asarray(wq_w, np.float32)
    Wfull[:, C_G:C_G + 1] = np.asarray(wg_w, np.float32)
    Wg = ln_g[:, None] * Wfull
    wg_bf = Wg.astype(ml_dtypes.bfloat16)
    s = wg_bf.astype(np.float32).sum(axis=0)                  # [128]
    # musrow holds the per-token MEAN (bn_aggr), so the rank-1 mu
    # correction needs the full column sum: z -= s_c * mu_t.
    negs = (-s)[None, :].astype(np.float32)

    bfull = np.zeros((P,), np.float32)
    bfull[C_WV:C_WV + R] = np.asarray(wv_b, np.float32)
    bfull[C_RQ:C_RQ + K] = np.asarray(rq_b, np.float32)
    bfull[C_WQ:C_WQ + K] = np.asarray(wq_b, np.float32)
    bfull[C_G] = np.float32(np.asarray(wg_b).reshape(-1)[0])
    bvec = bfull + ln_b @ Wfull                               # [128]
    wgb_eff = float(bvec[C_G])
    bvec_dev = bvec.copy()
    bvec_dev[C_G] = 0.0
    use_bias_vec = bool(np.any(np.abs(bvec_dev) > 0))

    rp_ext = np.concatenate(
        [mix * np.asarray(rp_w, np.float32),
         (mix * np.asarray(rp_b, np.float32))[None, :],
         np.zeros((1, D), np.float32)], axis=0)

    ident_f = np.eye(P, dtype=np.float32)
    selK0 = np.zeros((K, P), np.float32)
    selK1 = np.zeros((K, P), np.float32)
    for kk in range(4):
        for rr in range(R):
            selK0[kk, 32 * kk + rr] = 1.0
            selK1[kk + 4, 32 * kk + rr] = 1.0
    nselK0 = np.concatenate([-selK0, np.ones((1, P), np.float32)], axis=0)
    nselK1 = np.concatenate([-selK1, np.ones((1, P), np.float32)], axis=0)
    selR = np.zeros((R, P), np.float32)
    for kk in range(4):
        for rr in range(R):
            selR[rr, 32 * kk + rr] = 1.0
    selO = np.zeros((P, R + 2), np.float32)
    for kk in range(4):
        for rr in range(R):
            selO[32 * kk + rr, rr] = 1.0
    e32 = np.zeros((K, R + 2), np.float32)
    e32[:, R] = 1.0

    consts = {
        "wg": np.ascontiguousarray(wg_bf),
        "rp": np.ascontiguousarray(rp_ext),
        "negs": np.ascontiguousarray(negs),
        "ident_f": ident_f,
        "nselK0": nselK0, "nselK1": nselK1,
        "selK0": selK0, "selK1": selK1, "selR": selR, "selO": selO,
        "e32": e32,
        "onesP": np.ones((1, P), np.float32),
        "ones8c": np.concatenate([np.ones((K, 1), np.float32),
                                  np.zeros((K, 1), np.float32)], axis=1),
        "onesrow": np.ones((1, T), np.float32),
        "ones1x8": np.ones((1, K), np.float32),
        "bvec": np.ascontiguousarray(bvec_dev[:, None]),
    }

    x_bf = x.astype(ml_dtypes.bfloat16)

    nc = _get_program(wgb_eff, use_bias_vec)
    in_maps = []
    for c in range(NCORES):
        xx = x_bf[c]                                         # [T, D]
        # xbi[p, i, d] = x[i*128 + p, d]
        xbi = np.ascontiguousarray(
            xx.reshape(NT, P, D).transpose(1, 0, 2))
        # xTi[p, b, j, t] = x[b*512 + t, j*128 + p]
        xTi = np.ascontiguousarray(
            xx.reshape(NB, BLK, ND, P).transpose(3, 0, 2, 1))
        m = {"xbi": xbi, "xTi": xTi}
        m.update(consts)
        in_maps.append(m)

    res = run_bass_kernel_spmd(
        nc, in_maps, core_ids=list(range(NCORES)),
        trace=bool(int(os.environ.get("BASS_TRACE_RUN", "0"))))
    # y_perm[p, i, d] -> y[i*128 + p, d]
    out = np.stack(
        [r["y"].transpose(1, 0, 2).reshape(T, D) for r in res.results],
        axis=0)
    kernel.last_results = res
    return out


# revision 38
# speedup vs baseline: 1.2241x; 1.2241x over previous
"""Trainium2 Bass kernel for nn_ExplicitRegisters (scatter_memory), v3.

Reference math (per batch, L tokens, dim D, K heads, R registers):
    h   = LN(x) * g + b
    rw  = softmax(h @ rq_w + rq_b);  ww = softmax(h @ wq_w + wq_b)
    wv  = h @ wv_w + wv_b;           wg = sigmoid(h @ wg_w + wg_b)
    us  = ww * wg
    scan: rv_t = sum_k rw[t,k] regs[k,r]  (read before write)
          regs = (1-us_t) regs + us_t wv_t
    out = mix * (rv @ rp_w + rp_b)

v3 strategy (pure data parallel, one batch element per NeuronCore):
  - Host ships x twice in bf16, pre-swizzled so every input DMA is 128
    partitions x 8KB contiguous descriptors: token-major (LN stats) and
    d-major (matmul moving operand). Output goes to a permuted DRAM
    buffer and is unpermuted on the host.
  - Fully block-pipelined program (4 blocks of 512 tokens).
  - One padded bf16 matmul computes all projections channel-major:
    z^T rows [0:32)=wv, [32:40)=rq, [64:72)=wq, 96=gate.
  - LN folded post-matmul: bn_stats/accum stats, rank-1 mu correction
    on PE (bf16), rstd row broadcast via gpsimd partition_broadcast.
  - exp of the whole zlnT tile at once (bf16); selector matmuls read
    the 32-aligned rq/wq rows straight out of it.
  - alpha = sigmoid(gate)/sum_k E_w via reciprocal_approx_fast; all
    selector/replicate/output matmuls in bf16.
  - Recurrence: native tensor_tensor_scan over 256 (k,r) lanes in two
    [128, L] tiles.
"""

import os
import numpy as np
import ml_dtypes

import concourse.bacc as bacc
import concourse.bass as bass
import concourse.tile as tile
from concourse import mybir
from concourse.bass_utils import run_bass_kernel_spmd

B, L, D, K, R = 8, 2048, 1024, 8, 32
NCORES = 8
P = 128
T = L
NT = T // P            # 16 token tiles
NB = 4                 # blocks
BLK = 512
TPB = BLK // P         # 4 token tiles per block
ND = D // P            # 8 d-slices
EPS = 1e-5

# padded channel layout of the z matmul (partition bases must be 32-aligned)
C_WV = 0               # rows 0..31
C_RQ = 32              # rows 32..39
C_WQ = 64              # rows 64..71
C_G = 96               # row 96

f32 = mybir.dt.float32
f32r = mybir.dt.float32r
bf16 = mybir.dt.bfloat16
ts = bass.ts
AF = mybir.ActivationFunctionType
OP = mybir.AluOpType


def _r(ap):
    return ap if ap.dtype == f32r else ap.bitcast(f32r)


def build_program(wgb_eff: float, use_bias_vec: bool, debug: bool = False):
    nc = bacc.Bacc("TRN2", target_bir_lowering=False, debug=False,
                   enable_asserts=False, num_devices=NCORES)

    # pre-swizzled inputs: [P, ...] partition-major, 8KB/partition/block
    xbi_d = nc.dram_tensor("xbi", [P, NT, D], bf16, kind="ExternalInput")
    xTi_d = nc.dram_tensor("xTi", [P, NB, ND, BLK], bf16, kind="ExternalInput")
    wg_d = nc.dram_tensor("wg", [D, P], bf16, kind="ExternalInput")
    rp_d = nc.dram_tensor("rp", [R + 2, D], bf16, kind="ExternalInput")
    negs_d = nc.dram_tensor("negs", [1, P], f32r, kind="ExternalInput")
    ident_f_d = nc.dram_tensor("ident_f", [P, P], f32, kind="ExternalInput")
    ident_b_d = nc.dram_tensor("ident_b", [P, P], bf16, kind="ExternalInput")
    nselK0_d = nc.dram_tensor("nselK0", [K + 1, P], bf16, kind="ExternalInput")
    nselK1_d = nc.dram_tensor("nselK1", [K + 1, P], bf16, kind="ExternalInput")
    selK0_d = nc.dram_tensor("selK0", [K, P], bf16, kind="ExternalInput")
    selK1_d = nc.dram_tensor("selK1", [K, P], bf16, kind="ExternalInput")
    selR_d = nc.dram_tensor("selR", [R, P], f32r, kind="ExternalInput")
    selO_d = nc.dram_tensor("selO", [P, R + 2], bf16, kind="ExternalInput")
    e32_d = nc.dram_tensor("e32", [K, R + 2], bf16, kind="ExternalInput")
    ones8c_d = nc.dram_tensor("ones8c", [K, 2], bf16, kind="ExternalInput")
    onesP_d = nc.dram_tensor("onesP", [1, P], f32r, kind="ExternalInput")
    ones1x8_d = nc.dram_tensor("ones1x8", [1, K], f32r, kind="ExternalInput")
    onesrow_d = nc.dram_tensor("onesrow", [1, T], bf16, kind="ExternalInput")
    bvec_d = nc.dram_tensor("bvec", [P, 1], f32, kind="ExternalInput")
    # permuted output: y_perm[p, i, d] = y[i*128 + p, d]
    y_d = nc.dram_tensor("y", [P, NT, D], f32, kind="ExternalOutput")
    if debug:
        dbg = {
            "d_mv": nc.dram_tensor("d_mv", [P, NT, 2], f32, kind="ExternalOutput"),
            "d_musrow": nc.dram_tensor("d_musrow", [1, T], f32, kind="ExternalOutput"),
            "d_rstdrow": nc.dram_tensor("d_rstdrow", [1, T], f32, kind="ExternalOutput"),
            "d_zlnT": nc.dram_tensor("d_zlnT", [P, T], f32, kind="ExternalOutput"),
            "d_expZ": nc.dram_tensor("d_expZ", [P, T], bf16, kind="ExternalOutput"),
            "d_grow": nc.dram_tensor("d_grow", [1, T], f32, kind="ExternalOutput"),
            "d_usT": nc.dram_tensor("d_usT", [K + 1, T], bf16, kind="ExternalOutput"),
            "d_scan0": nc.dram_tensor("d_scan0", [P, T + 1], f32, kind="ExternalOutput"),
            "d_scan1": nc.dram_tensor("d_scan1", [P, T + 1], f32, kind="ExternalOutput"),
            "d_rvT": nc.dram_tensor("d_rvT", [R + 2, T], bf16, kind="ExternalOutput"),
            "d_rsr": nc.dram_tensor("d_rsr", [P, NT], f32, kind="ExternalOutput"),
        }

    yap = y_d.ap()

    with tile.TileContext(nc) as tc:
        with (
            tc.tile_pool(name="consts", bufs=1) as consts,
            tc.tile_pool(name="big", bufs=1) as big,
            tc.tile_pool(name="blk", bufs=2) as blk,
            tc.tile_pool(name="ps_z", bufs=1, space="PSUM") as ps_z,
            tc.tile_pool(name="ps_mm", bufs=2, space="PSUM") as ps_mm,
            tc.tile_pool(name="ps_pre", bufs=1, space="PSUM") as ps_pre,
            tc.tile_pool(name="ps_post", bufs=1, space="PSUM") as ps_post,
            tc.tile_pool(name="ps_rv", bufs=1, space="PSUM") as ps_rv,
            tc.tile_pool(name="ps_y", bufs=2, space="PSUM") as ps_y,
        ):
            # ---- persistent state ----
            xb_s = big.tile([P, NT, D], bf16)
            xT_s = big.tile([P, NB, ND, BLK], bf16)
            zlnT = big.tile([P, T], f32r)
            expZ = big.tile([P, T], bf16)
            er8 = big.tile([K, T], bf16)
            ew8 = big.tile([K, T], bf16)
            usT = big.tile([K + 1, T], bf16)     # row 8 = ones
            grow = big.tile([1, T], f32r)        # gate -> egn
            musrow = big.tile([1, T], f32r)
            rstdrow = big.tile([1, T], f32r)
            mv = big.tile([P, NT, 2], f32)       # (mean, var) per token
            scan0 = big.tile([P, T + 1], f32)
            scan1 = big.tile([P, T + 1], f32)
            rvT = big.tile([R + 2, T], bf16)
            rsr = big.tile([P, NT], f32)

            # ---- constants (wg first: z matmul needs it earliest) ----
            wg_s = consts.tile([P, ND, P], bf16)
            nc.sync.dma_start(out=wg_s,
                              in_=wg_d.ap().rearrange("(j p) c -> p j c", p=P))

            # block 0 inputs before the remaining consts
            nc.sync.dma_start(out=xT_s[:, 0], in_=xTi_d.ap()[:, 0])
            nc.sync.dma_start(out=xb_s[:, 0:TPB, :],
                              in_=xbi_d.ap()[:, 0:TPB, :])

            rp_s = consts.tile([R + 2, D], bf16)
            nc.sync.dma_start(out=rp_s, in_=rp_d.ap())
            ident_f = consts.tile([P, P], f32)
            nc.sync.dma_start(out=ident_f, in_=ident_f_d.ap())
            ident_b = consts.tile([P, P], bf16)
            nc.sync.dma_start(out=ident_b, in_=ident_b_d.ap())
            nselK0 = consts.tile([K + 1, P], bf16)
            nc.sync.dma_start(out=nselK0, in_=nselK0_d.ap())
            nselK1 = consts.tile([K + 1, P], bf16)
            nc.sync.dma_start(out=nselK1, in_=nselK1_d.ap())
            selK0 = consts.tile([K, P], bf16)
            nc.sync.dma_start(out=selK0, in_=selK0_d.ap())
            selK1 = consts.tile([K, P], bf16)
            nc.sync.dma_start(out=selK1, in_=selK1_d.ap())
            selR = consts.tile([R, P], f32r)
            nc.sync.dma_start(out=selR, in_=selR_d.ap())
            selO = consts.tile([P, R + 2], bf16)
            nc.sync.dma_start(out=selO, in_=selO_d.ap())
            e32 = consts.tile([K, R + 2], bf16)
            nc.sync.dma_start(out=e32, in_=e32_d.ap())
            negs = consts.tile([1, P], f32r)
            nc.sync.dma_start(out=negs, in_=negs_d.ap())
            ones8c = consts.tile([K, 2], bf16)
            nc.sync.dma_start(out=ones8c, in_=ones8c_d.ap())
            onesP = consts.tile([1, P], f32r)
            nc.sync.dma_start(out=onesP, in_=onesP_d.ap())
            ones1x8 = consts.tile([1, K], f32r)
            nc.sync.dma_start(out=ones1x8, in_=ones1x8_d.ap())
            bvec = consts.tile([P, 1], f32)
            nc.sync.dma_start(out=bvec, in_=bvec_d.ap())
            epsb = consts.tile([P, 1], f32)
            nc.vector.memset(epsb, EPS)
            gbias = consts.tile([P, 1], f32)
            nc.vector.memset(gbias, -wgb_eff)

            nc.vector.memset(scan0[:, 0:1], 0.0)
            nc.vector.memset(scan1[:, 0:1], 0.0)
            nc.sync.dma_start(out=usT[K:K + 1, :], in_=onesrow_d.ap())

            # remaining input blocks
            for b in range(1, NB):
                nc.sync.dma_start(out=xT_s[:, b], in_=xTi_d.ap()[:, b])
                nc.sync.dma_start(out=xb_s[:, b * TPB:(b + 1) * TPB, :],
                                  in_=xbi_d.ap()[:, b * TPB:(b + 1) * TPB, :])

            # ---- per-block pipeline ----
            for b in range(NB):
                # -- stats: 2 tiles DVE bn_stats, 2 tiles ACT copy/square --
                for ii in range(TPB):
                    i = b * TPB + ii
                    xi = xb_s[:, i, :]
                    if ii < 2:
                        st6 = blk.tile([P, 2, 6], f32, tag="st6")
                        xig = xi.rearrange("p (g f) -> p g f", f=512)
                        nc.vector.bn_stats(st6[:, 0, :], xig[:, 0, :])
                        nc.vector.bn_stats(st6[:, 1, :], xig[:, 1, :])
                        nc.vector.bn_aggr(mv[:, i, :], st6)
                    else:
                        sc = blk.tile([P, D], bf16, tag="sc")
                        scol = blk.tile([P, 2], f32, tag="scol")
                        nc.scalar.activation(sc, xi, AF.Copy,
                                             accum_out=scol[:, 0:1])
                        nc.scalar.activation(sc, xi, AF.Square,
                                             accum_out=scol[:, 1:2])
                        nc.vector.tensor_scalar(
                            out=mv[:, i, 0:1], in0=scol[:, 0:1],
                            scalar1=1.0 / D, scalar2=None, op0=OP.mult)
                        msq = blk.tile([P, 1], f32, tag="msq")
                        nc.vector.tensor_mul(msq, mv[:, i, 0:1], mv[:, i, 0:1])
                        nc.vector.scalar_tensor_tensor(
                            out=mv[:, i, 1:2], in0=scol[:, 1:2],
                            scalar=1.0 / D, in1=msq,
                            op0=OP.mult, op1=OP.subtract)

                # -- z matmul (held open for the mu correction) --
                zp = ps_z.tile([P, BLK], f32, tag="zp")
                for j in range(ND):
                    nc.tensor.matmul(zp, wg_s[:, j, :],
                                     xT_s[:, b, j, :],
                                     start=(j == 0), stop=False)

                # -- stats tail: column math + row-form conversion --
                pack = blk.tile([P, 2 * TPB], f32, tag="pack")
                nc.vector.tensor_copy(pack[:, 0:TPB],
                                      mv[:, b * TPB:(b + 1) * TPB, 0])
                lnv = blk.tile([P, TPB], f32, tag="lnv")
                nc.scalar.activation(lnv, mv[:, b * TPB:(b + 1) * TPB, 1],
                                     AF.Ln, bias=epsb)
                nc.scalar.activation(pack[:, TPB:2 * TPB], lnv, AF.Exp,
                                     scale=-0.5)
                pkT = ps_pre.tile([2 * TPB, P], f32, tag="pre")
                nc.tensor.transpose(pkT, pack, ident_f)
                pks = blk.tile([2 * TPB, P], f32r, tag="pks")
                nc.vector.tensor_copy(pks, pkT)
                nc.gpsimd.dma_start(
                    out=musrow[:, ts(b, BLK)].rearrange(
                        "o (c p) -> o c p", p=P),
                    in_=pks[0:TPB, :])
                nc.gpsimd.dma_start(
                    out=rstdrow[:, ts(b, BLK)].rearrange(
                        "o (c p) -> o c p", p=P),
                    in_=pks[TPB:2 * TPB, :])

                # -- LN fold: rank-1 mu correction + rstd scale --
                nc.tensor.matmul(
                    zp, _r(negs), _r(musrow[:, ts(b, BLK)]),
                    start=False, stop=True, skip_group_check=True)
                rr = ps_mm.tile([P, BLK], f32, tag="mm")
                nc.tensor.matmul(rr, _r(onesP), _r(rstdrow[:, ts(b, BLK)]),
                                 start=True, stop=True)
                rstdrep = blk.tile([P, BLK], f32, tag="rstdrep")
                nc.scalar.copy(rstdrep, rr)
                nc.vector.tensor_mul(zlnT[:, ts(b, BLK)], zp, rstdrep)
                if use_bias_vec:
                    nc.vector.tensor_scalar(
                        out=zlnT[:, ts(b, BLK)], in0=zlnT[:, ts(b, BLK)],
                        scalar1=bvec[:, 0:1], scalar2=None, op0=OP.add)

                # -- exp of the whole tile; gate row handled separately --
                nc.gpsimd.dma_start(out=grow[:, ts(b, BLK)],
                                    in_=zlnT[C_G:C_G + 1, ts(b, BLK)])
                nc.scalar.activation(expZ[:, ts(b, BLK)], zlnT[:, ts(b, BLK)],
                                     AF.Exp)
                nc.gpsimd.dma_start(out=ew8[:, ts(b, BLK)],
                                    in_=expZ[C_WQ:C_WQ + K, ts(b, BLK)])
                nc.gpsimd.dma_start(out=er8[:, ts(b, BLK)],
                                    in_=expZ[C_RQ:C_RQ + K, ts(b, BLK)])
                # egn = exp(-(g + wgb))
                nc.scalar.activation(grow[:, ts(b, BLK)], grow[:, ts(b, BLK)],
                                     AF.Exp, scale=-1.0, bias=gbias[0:1, 0:1])

                # -- alpha = sigmoid(g)/sum_k E_w, to 8 rows --
                swp = ps_pre.tile([2, BLK], f32, tag="pre")
                nc.tensor.matmul(swp, ones8c, ew8[:, ts(b, BLK)],
                                 start=True, stop=True)
                deno = blk.tile([1, BLK], f32r, tag="deno")
                nc.vector.scalar_tensor_tensor(
                    out=deno, in0=grow[:, ts(b, BLK)], scalar=1.0,
                    in1=swp[0:1, :], op0=OP.add, op1=OP.mult)
                den8 = ps_pre.tile([K, BLK], f32, tag="pre")
                nc.tensor.matmul(den8, _r(ones1x8), _r(deno),
                                 start=True, stop=True)
                rd8 = blk.tile([K, BLK], f32, tag="rd8")
                nc.vector.reciprocal_approx_fast(rd8, den8)
                nc.vector.tensor_mul(usT[0:K, ts(b, BLK)],
                                     ew8[:, ts(b, BLK)], rd8)

                # -- replicate + scan --
                for kt, (nselK, selK, scanb) in enumerate(
                        [(nselK0, selK0, scan0), (nselK1, selK1, scan1)]):
                    d0p = ps_mm.tile([P, BLK], f32, tag="mm")
                    nc.tensor.matmul(d0p, nselK, usT[:, ts(b, BLK)],
                                     start=True, stop=True)
                    wrep = ps_mm.tile([P, BLK], f32, tag="mm")
                    nc.tensor.matmul(wrep, selR,
                                     _r(zlnT[C_WV:C_WV + R, ts(b, BLK)]),
                                     start=True, stop=True)
                    wreps = blk.tile([P, BLK], f32, tag="wreps")
                    nc.scalar.copy(wreps, wrep)
                    urep = ps_mm.tile([P, BLK], f32, tag="mm")
                    nc.tensor.matmul(urep, selK,
                                     usT[0:K, ts(b, BLK)],
                                     start=True, stop=True)
                    d1 = blk.tile([P, BLK], f32, tag="d1")
                    nc.vector.tensor_mul(d1, urep, wreps)
                    nc.vector.tensor_tensor_scan(
                        out=scanb[:, 1 + b * BLK:1 + (b + 1) * BLK],
                        data0=d0p, data1=d1,
                        initial=scanb[:, b * BLK:b * BLK + 1],
                        op0=OP.mult, op1=OP.add)

                # -- read side --
                rvp = ps_rv.tile([R + 2, BLK], f32, tag="rv")
                for kt, (selK, scanb) in enumerate(
                        [(selK0, scan0), (selK1, scan1)]):
                    erep = ps_mm.tile([P, BLK], f32, tag="mm")
                    nc.tensor.matmul(erep, selK, er8[:, ts(b, BLK)],
                                     start=True, stop=True)
                    rtmp = blk.tile([P, BLK], bf16, tag="rtmp")
                    nc.vector.tensor_mul(
                        rtmp, scanb[:, ts(b, BLK)], erep)
                    nc.tensor.matmul(rvp, selO, rtmp,
                                     start=(kt == 0), stop=False)
                nc.tensor.matmul(rvp, e32, er8[:, ts(b, BLK)],
                                 start=False, stop=True)
                nc.scalar.copy(rvT[:, ts(b, BLK)], rvp)

                # -- read normalization columns: 1/sum_k E_r --
                srt = blk.tile([TPB, P], bf16, tag="srt")
                nc.gpsimd.dma_start(
                    out=srt,
                    in_=rvT[R:R + 1, ts(b, BLK)].rearrange(
                        "o (c p) -> o c p", p=P))
                srtp = ps_post.tile([P, TPB], bf16, tag="post")
                nc.tensor.transpose(srtp, srt, ident_b[0:TPB, 0:TPB])
                nc.vector.reciprocal(rsr[:, b * TPB:(b + 1) * TPB], srtp)

                # -- output projection --
                for hh in range(2):
                    ys = blk.tile([P, 2, D], f32, tag="ys")
                    for ii in range(2):
                        i = b * TPB + hh * 2 + ii
                        for half in range(2):
                            yp = ps_y.tile([P, BLK], f32, tag="y")
                            nc.tensor.matmul(yp, rvT[:, ts(i, P)],
                                             rp_s[:, ts(half, BLK)],
                                             start=True, stop=True)
                            if ii == 0 and half == 0:
                                nc.vector.tensor_scalar(
                                    out=ys[:, ii, ts(half, BLK)], in0=yp,
                                    scalar1=rsr[:, i:i + 1], scalar2=None,
                                    op0=OP.mult)
                            else:
                                nc.scalar.activation(
                                    ys[:, ii, ts(half, BLK)], yp, AF.Copy,
                                    scale=rsr[:, i:i + 1])
                    nc.sync.dma_start(
                        out=yap[:, b * TPB + hh * 2:b * TPB + hh * 2 + 2, :],
                        in_=ys)

            if debug:
                nc.sync.dma_start(out=dbg["d_mv"].ap(), in_=mv)
                nc.sync.dma_start(out=dbg["d_musrow"].ap(), in_=musrow.bitcast(f32))
                nc.sync.dma_start(out=dbg["d_rstdrow"].ap(), in_=rstdrow.bitcast(f32))
                nc.sync.dma_start(out=dbg["d_zlnT"].ap(), in_=zlnT.bitcast(f32))
                nc.sync.dma_start(out=dbg["d_expZ"].ap(), in_=expZ)
                nc.sync.dma_start(out=dbg["d_grow"].ap(), in_=grow.bitcast(f32))
                nc.sync.dma_start(out=dbg["d_usT"].ap(), in_=usT)
                nc.sync.dma_start(out=dbg["d_scan0"].ap(), in_=scan0)
                nc.sync.dma_start(out=dbg["d_scan1"].ap(), in_=scan1)
                nc.sync.dma_start(out=dbg["d_rvT"].ap(), in_=rvT)
                nc.sync.dma_start(out=dbg["d_rsr"].ap(), in_=rsr)

    nc.finalize()
    return nc


_CACHE = {}


def _get_program(wgb_eff: float, use_bias_vec: bool):
    dbgflag = bool(int(os.environ.get("KERNEL_DEBUG", "0")))
    key = (round(float(wgb_eff), 8), use_bias_vec, dbgflag)
    if key not in _CACHE:
        _CACHE[key] = build_program(wgb_eff, use_bias_vec, debug=dbgflag)
    return _CACHE[key]


def kernel(x, ln_g, ln_b, rq_w, rq_b, rp_w, rp_b, wq_w, wq_b, wv_w, wv_b,
           wg_w, wg_b, mix, **_unused):
    x = np.asarray(x, np.float32)
    ln_g = np.asarray(ln_g, np.float32)
    ln_b = np.asarray(ln_b, np.float32)
    mix = np.float32(np.asarray(mix))

    Wfull = np.zeros((D, P), np.float32)
    Wfull[:, C_WV:C_WV + R] = np.asarray(wv_w, np.float32)
    Wfull[:, C_RQ:C_RQ + K] = np.asarray(rq_w, np.float32)
    Wfull[:, C_WQ:C_WQ + K] = np.asarray(wq_w, np.float32)
    Wfull[:, C_G:C_G + 1] = np.asarray(wg_w, np.float32)
    Wg = ln_g[:, None] * Wfull
    wg_bf = Wg.astype(ml_dtypes.bfloat16)
    s = wg_bf.astype(np.float32).sum(axis=0)                  # [128]
    # musrow holds the per-token MEAN (bn_aggr), so the rank-1 mu
    # correction needs the full column sum: z -= s_c * mu_t.
    negs = (-s)[None, :].astype(np.float32)

    bfull = np.zeros((P,), np.float32)
    bfull[C_WV:C_WV + R] = np.asarray(wv_b, np.float32)
    bfull[C_RQ:C_RQ + K] = np.asarray(rq_b, np.float32)
    bfull[C_WQ:C_WQ + K] = np.asarray(wq_b, np.float32)
    bfull[C_G] = np.float32(np.asarray(wg_b).reshape(-1)[0])
    bvec = bfull + ln_b @ Wfull                               # [128]
    wgb_eff = float(bvec[C_G])
    bvec_dev = bvec.copy()
    bvec_dev[C_G] = 0.0
    use_bias_vec = bool(np.any(np.abs(bvec_dev) > 0))

    rp_ext = np.concatenate(
        [mix * np.asarray(rp_w, np.float32),
         (mix * np.asarray(rp_b, np.float32))[None, :],
         np.zeros((1, D), np.float32)], axis=0)

    ident_f = np.eye(P, dtype=np.float32)
    ident_b = np.eye(P, dtype=np.float32).astype(ml_dtypes.bfloat16)
    selK0 = np.zeros((K, P), np.float32)
    selK1 = np.zeros((K, P), np.float32)
    for kk in range(4):
        for rr in range(R):
            selK0[kk, 32 * kk + rr] = 1.0
            selK1[kk + 4, 32 * kk + rr] = 1.0
    nselK0 = np.concatenate([-selK0, np.ones((1, P), np.float32)], axis=0)
    nselK1 = np.concatenate([-selK1, np.ones((1, P), np.float32)], axis=0)
    selR = np.zeros((R, P), np.float32)
    for kk in range(4):
        for rr in range(R):
            selR[rr, 32 * kk + rr] = 1.0
    selO = np.zeros((P, R + 2), np.float32)
    for kk in range(4):
        for rr in range(R):
            selO[32 * kk + rr, rr] = 1.0
    e32 = np.zeros((K, R + 2), np.float32)
    e32[:, R] = 1.0

    bf = ml_dtypes.bfloat16
    consts = {
        "wg": np.ascontiguousarray(wg_bf),
        "rp": np.ascontiguousarray(rp_ext.astype(bf)),
        "negs": np.ascontiguousarray(negs),
        "ident_f": ident_f,
        "ident_b": ident_b,
        "nselK0": nselK0.astype(bf), "nselK1": nselK1.astype(bf),
        "selK0": selK0.astype(bf), "selK1": selK1.astype(bf),
        "selR": selR, "selO": selO.astype(bf),
        "e32": e32.astype(bf),
        "ones8c": np.concatenate([np.ones((K, 1), np.float32),
                                  np.zeros((K, 1), np.float32)],
                                 axis=1).astype(bf),
        "onesrow": np.ones((1, T), np.float32).astype(bf),
        "onesP": np.ones((1, P), np.float32),
        "ones1x8": np.ones((1, K), np.float32),
        "bvec": np.ascontiguousarray(bvec_dev[:, None]),
    }

    x_bf = x.astype(bf)

    nc = _get_program(wgb_eff, use_bias_vec)
    in_maps = []
    for c in range(NCORES):
        xx = x_bf[c]                                         # [T, D]
        # xbi[p, i, d] = x[i*128 + p, d]
        xbi = np.ascontiguousarray(
            xx.reshape(NT, P, D).transpose(1, 0, 2))
        # xTi[p, b, j, t] = x[b*512 + t, j*128 + p]
        xTi = np.ascontiguousarray(
            xx.reshape(NB, BLK, ND, P).transpose(3, 0, 2, 1))
        m = {"xbi": xbi, "xTi": xTi}
        m.update(consts)
        in_maps.append(m)

    res = run_bass_kernel_spmd(
        nc, in_maps, core_ids=list(range(NCORES)),
        trace=bool(int(os.environ.get("BASS_TRACE_RUN", "0"))))
    # y_perm[p, i, d] -> y[i*128 + p, d]
    out = np.stack(
        [r["y"].transpose(1, 0, 2).reshape(T, D) for r in res.results],
        axis=0)
    kernel.last_results = res
    return out
